# revision 1
# baseline (speedup 1.0000x reference)
"""Trainium2 Bass kernel for nn_LinkPredictor (2-layer GCN + edge-dot decode).

Strategy (8 NeuronCores, SPMD):
  - Nodes sharded: core c owns rows [c*12544, (c+1)*12544) of the padded
    node table (N=100000 padded to 100352 = 8*98*128).
  - Edges assigned to the core owning their dst. Per core, edges are grouped
    by (dst window of 128 nodes, src chunk of 25088 rows) with a uniform slot
    budget B per group (padded with dead slots, norm=0) so all 8 cores run an
    identical program.
  - GCN norm (dinv[s]*dinv[d]) is folded into one-hot selection matrices S
    built on-device by a dual-op tensor_scalar: S[e,:] = (iota==dstloc[e])*norm[e].
  - Message gather: dma_gather (GPSIMD SWDGE, 4 queues) from a bf16 node table
    in DRAM; segment-sum = PE matmul accumulation S^T @ M into PSUM (fp32).
  - Self-loops handled separately (dinv^2 * h[n], no gather).
  - Inter-layer full-table exchange via AllGather collectives.
  - Decode: gather z2[s], z2[d], DVE multiply + reduce.
"""
import contextlib
import math
import numpy as np
import ml_dtypes

import concourse.bass as bass
import concourse.tile as tile
from concourse import bacc, mybir
from concourse.bass_utils import run_bass_kernel_spmd
from concourse.tile_rust import add_dep_helper

F32 = mybir.dt.float32
BF16 = mybir.dt.bfloat16
I16 = mybir.dt.int16
BF = ml_dtypes.bfloat16


class Cfg:
    def __init__(self, N=100000, E=1600000, EL=100000, D=128, ncores=8,
                 nw=98, nchunks=4, wb=4):
        self.N, self.E, self.EL, self.D, self.NC = N, E, EL, D, ncores
        self.NW = nw                      # windows (128 nodes each) per core
        self.SHARD = nw * 128             # nodes per core (padded)
        self.NP = self.SHARD * ncores     # padded node count
        assert self.NP >= N
        self.NCH = nchunks                # src chunks (int16 index range)
        assert self.NP % nchunks == 0
        self.CHROWS = self.NP // nchunks
        assert self.CHROWS <= 32768
        self.WB = wb                      # windows per gather/aggregate batch
        self.NBATCH = math.ceil(nw / wb)


DEFAULT = Cfg()


def _wrap_idxs(idx):
    """[n] ints -> [128, n//16] int16 wrapped in 16 partitions, replicated 8x."""
    n = len(idx)
    assert n % 16 == 0
    w = np.asarray(idx, dtype=np.int16).reshape(n // 16, 16).T
    return np.ascontiguousarray(np.tile(w, (8, 1)))


def host_prep(cfg, x, edge_index, edge_label_index, W1, b1, W2, b2):
    """All host-side sharding/layout. Returns (per-core input maps, meta)."""
    c = cfg
    src = np.asarray(edge_index[0], dtype=np.int64)
    dst = np.asarray(edge_index[1], dtype=np.int64)
    deg = np.bincount(dst, minlength=c.N).astype(np.float64) + 1.0
    dinv = 1.0 / np.sqrt(deg)                      # [N]
    dinv_p = np.ones(c.NP, dtype=np.float64)
    dinv_p[:c.N] = dinv
    norm_e = (dinv[src] * dinv[dst]).astype(np.float32)

    core_of = dst // c.SHARD
    w_of = (dst - core_of * c.SHARD) // 128
    ch_of = src // c.CHROWS

    key = (core_of * c.NW + w_of) * c.NCH + ch_of
    order = np.argsort(key, kind="stable")
    ngroups = c.NC * c.NW * c.NCH
    counts = np.bincount(key[order], minlength=ngroups)
    B = int(128 * math.ceil(max(int(counts.max()), 1) / 128))
    starts = np.zeros(ngroups + 1, dtype=np.int64)
    np.cumsum(counts, out=starts[1:])

    TPG = B // 128                       # tiles per group
    TOT = c.NW * c.NCH * B               # slots per core per layer
    TOT_TILES = TOT // 128

    # global slot order per core: batch b -> chunk ch -> window w (in batch) -> i
    idx_arr = np.zeros((c.NC, TOT), dtype=np.int64)
    dstloc_arr = np.zeros((c.NC, TOT), dtype=np.float32)
    norm_arr = np.zeros((c.NC, TOT), dtype=np.float32)
    for core in range(c.NC):
        pos = 0
        for b in range(c.NBATCH):
            wlo, whi = b * c.WB, min((b + 1) * c.WB, c.NW)
            for ch in range(c.NCH):
                for w in range(wlo, whi):
                    g = (core * c.NW + w) * c.NCH + ch
                    eids = order[starts[g]:starts[g + 1]]
                    n = len(eids)
                    idx_arr[core, pos:pos + n] = src[eids] - ch * c.CHROWS
                    dstloc_arr[core, pos:pos + n] = (
                        dst[eids] - core * c.SHARD - w * 128)
                    norm_arr[core, pos:pos + n] = norm_e[eids]
                    pos += B
        assert pos == TOT

    # decode: label edge j -> core j // ELC; slots grouped by (chunk(s), chunk(d))
    assert c.EL % c.NC == 0
    ELC = c.EL // c.NC
    ls = np.asarray(edge_label_index[0], dtype=np.int64)
    ld = np.asarray(edge_label_index[1], dtype=np.int64)
    kd = (ls // c.CHROWS) * c.NCH + (ld // c.CHROWS)
    NG_DEC = c.NCH * c.NCH
    B_dec = 0
    for core in range(c.NC):
        cnt = np.bincount(kd[core * ELC:(core + 1) * ELC], minlength=NG_DEC)
        B_dec = max(B_dec, int(cnt.max()))
    B_dec = 128 * math.ceil(max(B_dec, 1) / 128)
    TOT_DEC = NG_DEC * B_dec
    idx_s = np.zeros((c.NC, TOT_DEC), dtype=np.int64)
    idx_d = np.zeros((c.NC, TOT_DEC), dtype=np.int64)
    slot2j = np.full((c.NC, TOT_DEC), -1, dtype=np.int64)
    for core in range(c.NC):
        jlo = core * ELC
        kk = kd[jlo:jlo + ELC]
        o = np.argsort(kk, kind="stable")
        cnt = np.bincount(kk, minlength=NG_DEC)
        st = np.zeros(NG_DEC + 1, dtype=np.int64)
        np.cumsum(cnt, out=st[1:])
        for g in range(NG_DEC):
            js = o[st[g]:st[g + 1]] + jlo
            n = len(js)
            pos = g * B_dec
            idx_s[core, pos:pos + n] = ls[js] - (g // c.NCH) * c.CHROWS
            idx_d[core, pos:pos + n] = ld[js] - (g % c.NCH) * c.CHROWS
            slot2j[core, pos:pos + n] = js

    xp = np.zeros((c.NP, c.D), dtype=np.float32)
    xp[:c.N] = np.asarray(x, dtype=np.float32)
    dinv_f = dinv_p.astype(np.float32)
    in_maps = []
    for core in range(c.NC):
        sl = slice(core * c.SHARD, (core + 1) * c.SHARD)
        in_maps.append({
            "xT": np.ascontiguousarray(xp[sl].T).astype(BF),
            "W1": np.asarray(W1, dtype=np.float32).astype(BF),
            "W2": np.asarray(W2, dtype=np.float32).astype(BF),
            "b1r": np.tile(np.asarray(b1, np.float32)[None, :], (128, 1)),
            "b2r": np.tile(np.asarray(b2, np.float32)[None, :], (128, 1)),
            "dinv2": np.ascontiguousarray(
                (dinv_f[sl] ** 2).reshape(c.NW, 128).T),
            "gidx": _wrap_idxs(idx_arr[core]),
            "dstloc": np.ascontiguousarray(
                dstloc_arr[core].reshape(TOT_TILES, 128).T),
            "gnorm": np.ascontiguousarray(
                norm_arr[core].reshape(TOT_TILES, 128).T),
            "didx_s": _wrap_idxs(idx_s[core]),
            "didx_d": _wrap_idxs(idx_d[core]),
        })
    meta = dict(B=B, TPG=TPG, TOT=TOT, TOT_TILES=TOT_TILES,
                B_dec=B_dec, TOT_DEC=TOT_DEC, slot2j=slot2j)
    return in_maps, meta


def build_program(cfg, meta, num_cores=None):
    c = cfg
    NCores = num_cores or c.NC
    B, TPG, TOT, TOT_TILES = meta["B"], meta["TPG"], meta["TOT"], meta["TOT_TILES"]
    B_dec, TOT_DEC = meta["B_dec"], meta["TOT_DEC"]
    D = c.D

    nc = bacc.Bacc("TRN2", target_bir_lowering=False, debug=False,
                   num_devices=NCores, num_swdge_queues=min(4, c.NCH))
    NQ = min(4, c.NCH)

    xT_in = nc.dram_tensor("xT", [D, c.SHARD], BF16, kind="ExternalInput")
    W1_in = nc.dram_tensor("W1", [D, D], BF16, kind="ExternalInput")
    W2_in = nc.dram_tensor("W2", [D, D], BF16, kind="ExternalInput")
    b1_in = nc.dram_tensor("b1r", [128, D], F32, kind="ExternalInput")
    b2_in = nc.dram_tensor("b2r", [128, D], F32, kind="ExternalInput")
    dinv2_in = nc.dram_tensor("dinv2", [128, c.NW], F32, kind="ExternalInput")
    gidx_in = nc.dram_tensor("gidx", [128, TOT // 16], I16, kind="ExternalInput")
    dstloc_in = nc.dram_tensor("dstloc", [128, TOT_TILES], F32, kind="ExternalInput")
    gnorm_in = nc.dram_tensor("gnorm", [128, TOT_TILES], F32, kind="ExternalInput")
    didx_s_in = nc.dram_tensor("didx_s", [128, TOT_DEC // 16], I16, kind="ExternalInput")
    didx_d_in = nc.dram_tensor("didx_d", [128, TOT_DEC // 16], I16, kind="ExternalInput")
    dots_out = nc.dram_tensor("dots", [128, TOT_DEC // 128], F32, kind="ExternalOutput")

    gst = {"count": 0, "prev": None}

    def emit_gather(out_ap, in_ap, idx_ap, n_idx):
        q = gst["count"] % NQ
        inst = nc.gpsimd.dma_gather(out_ap, in_ap, idx_ap, n_idx, n_idx, D,
                                    queue_num=q, single_packet=False)
        if gst["prev"] is not None:
            add_dep_helper(inst.ins, gst["prev"].ins, sync=False,
                           reason="pin swdge queue order")
        gst["prev"] = inst
        gst["count"] += 1
        return inst

    shard1 = nc.dram_tensor("shard1", [c.SHARD, D], BF16)
    shard2 = nc.dram_tensor("shard2", [c.SHARD, D], BF16)
    shardz = nc.dram_tensor("shardz", [c.SHARD, D], BF16)
    table1 = nc.dram_tensor("table1", [c.NP, D], BF16)
    table2 = nc.dram_tensor("table2", [c.NP, D], BF16)
    tablez = nc.dram_tensor("tablez", [c.NP, D], BF16)

    iota_dram = nc.inline_tensor(
        np.tile(np.arange(128, dtype=np.float32), (128, 1)).astype(BF), "iota_c")
    ident_dram = nc.inline_tensor(np.eye(128, dtype=np.float32).astype(BF), "ident_c")

    cc_sem = nc.alloc_semaphore("cc_sem")
    core_ids = list(range(NCores))

    with tile.TileContext(nc) as tc:
        with contextlib.ExitStack() as es:
            const = es.enter_context(tc.tile_pool(name="const", bufs=1))
            meta_p = es.enter_context(tc.tile_pool(name="meta", bufs=1))

            w1_sb = const.tile([D, D], BF16); nc.sync.dma_start(w1_sb[:], W1_in[:])
            w2_sb = const.tile([D, D], BF16); nc.sync.dma_start(w2_sb[:], W2_in[:])
            b1_sb = const.tile([128, D], F32); nc.sync.dma_start(b1_sb[:], b1_in[:])
            b2_sb = const.tile([128, D], F32); nc.sync.dma_start(b2_sb[:], b2_in[:])
            dinv2_sb = const.tile([128, c.NW], F32)
            nc.sync.dma_start(dinv2_sb[:], dinv2_in[:])
            iota_sb = const.tile([128, 128], BF16)
            nc.sync.dma_start(iota_sb[:], iota_dram[:])
            ident_sb = const.tile([128, 128], BF16)
            nc.sync.dma_start(ident_sb[:], ident_dram[:])
            gidx_sb = meta_p.tile([128, TOT // 16], I16)
            nc.sync.dma_start(gidx_sb[:], gidx_in[:])
            dstloc_sb = meta_p.tile([128, TOT_TILES], F32)
            nc.sync.dma_start(dstloc_sb[:], dstloc_in[:])
            gnorm_sb = meta_p.tile([128, TOT_TILES], F32)
            nc.sync.dma_start(gnorm_sb[:], gnorm_in[:])

            def all_gather(shard, table_out, sem, n_before):
                tc.strict_bb_all_engine_barrier()
                with tc.tile_critical():
                    nc.gpsimd.collective_compute(
                        "AllGather", mybir.AluOpType.bypass,
                        replica_groups=[core_ids],
                        ins=[shard[:]], outs=[table_out[:]],
                    ).then_inc(cc_sem)
                    nc.gpsimd.wait_ge(cc_sem, n_before + 1)
                tc.strict_bb_all_engine_barrier()

            def layer(lid, table, h_tiles, bias_sb, shard_next, sem_next,
                      out_pool, make_next):
                out_tiles = []
                with tc.tile_pool(name=f"M{lid}", bufs=2) as Mp, \
                     tc.tile_pool(name=f"S{lid}", bufs=4) as Sp, \
                     tc.tile_pool(name=f"ag{lid}", bufs=4, space="PSUM") as agp, \
                     tc.tile_pool(name=f"tp{lid}", bufs=2, space="PSUM") as tpp, \
                     tc.tile_pool(name=f"ep{lid}", bufs=3) as epp:
                    for b in range(c.NBATCH):
                        wlo = b * c.WB
                        whi = min(wlo + c.WB, c.NW)
                        nwb = whi - wlo
                        cols_per_ch = nwb * TPG
                        Mt = Mp.tile([128, c.NCH * cols_per_ch, D], BF16, tag="M")
                        slot_base = wlo * c.NCH * B
                        for ch in range(c.NCH):
                            n_idx = nwb * B
                            off16 = (slot_base + ch * n_idx) // 16
                            emit_gather(
                                Mt[:, ch * cols_per_ch:(ch + 1) * cols_per_ch, :],
                                table[ch * c.CHROWS:(ch + 1) * c.CHROWS, :],
                                gidx_sb[:, off16:off16 + n_idx // 16],
                                n_idx)
                        tile_base = slot_base // 128
                        for wi in range(nwb):
                            w = wlo + wi
                            ps = agp.tile([128, D], F32, tag="agg")
                            nmm = c.NCH * TPG
                            k = 0
                            for ch in range(c.NCH):
                                for t in range(TPG):
                                    tcol = tile_base + (ch * nwb + wi) * TPG + t
                                    S = Sp.tile([128, 128], BF16, tag="S")
                                    nc.vector.tensor_scalar(
                                        S[:], iota_sb[:],
                                        dstloc_sb[:, tcol:tcol + 1],
                                        gnorm_sb[:, tcol:tcol + 1],
                                        mybir.AluOpType.is_equal,
                                        mybir.AluOpType.mult)
                                    mcol = (ch * nwb + wi) * TPG + t
                                    nc.tensor.matmul(
                                        ps[:], lhsT=S[:], rhs=Mt[:, mcol, :],
                                        start=(k == 0), stop=(k == nmm - 1))
                                    k += 1
                            selfz = epp.tile([128, D], F32, tag="selfz")
                            nc.vector.tensor_scalar(
                                selfz[:], h_tiles[w][:],
                                dinv2_sb[:, w:w + 1], None,
                                mybir.AluOpType.mult)
                            s1 = epp.tile([128, D], F32, tag="s1")
                            nc.vector.tensor_tensor(
                                s1[:], ps[:], selfz[:], op=mybir.AluOpType.add)
                            s2 = epp.tile([128, D], F32, tag="s2")
                            nc.vector.tensor_tensor(
                                s2[:], s1[:], bias_sb[:], op=mybir.AluOpType.add)
                            if make_next:
                                z = epp.tile([128, D], BF16, tag="z")
                                nc.scalar.activation(
                                    z[:], s2[:], mybir.ActivationFunctionType.Relu)
                                zt_ps = tpp.tile([128, D], BF16, tag="zt")
                                nc.tensor.transpose(zt_ps[:], z[:], ident_sb[:])
                                zT = epp.tile([128, D], BF16, tag="zT")
                                nc.vector.tensor_copy(zT[:], zt_ps[:])
                                h2ps = tpp.tile([128, D], F32, tag="h2")
                                nc.tensor.matmul(h2ps[:], lhsT=zT[:], rhs=w2_sb[:],
                                                 start=True, stop=True)
                                ht = out_pool.tile([128, D], BF16, tag="nxt")
                                nc.vector.tensor_copy(ht[:], h2ps[:])
                            else:
                                ht = out_pool.tile([128, D], BF16, tag="nxt")
                                nc.scalar.activation(
                                    ht[:], s2[:], mybir.ActivationFunctionType.Relu)
                            nc.sync.dma_start(
                                shard_next[w * 128:(w + 1) * 128, :], ht[:])
                            out_tiles.append(ht)
                return out_tiles

            with tc.tile_pool(name="hsb2", bufs=c.NW) as hsb2:
                with tc.tile_pool(name="hsb1", bufs=c.NW) as hsb1:
                    # P0: h1 = x @ W1 for own shard
                    h1_tiles = []
                    with tc.tile_pool(name="p0", bufs=3) as p0, \
                         tc.tile_pool(name="p0ps", bufs=2, space="PSUM") as p0ps:
                        for w in range(c.NW):
                            xt = p0.tile([D, 128], BF16)
                            nc.sync.dma_start(
                                xt[:], xT_in[:, w * 128:(w + 1) * 128])
                            ps = p0ps.tile([128, D], F32, tag="ps")
                            nc.tensor.matmul(ps[:], lhsT=xt[:], rhs=w1_sb[:],
                                             start=True, stop=True)
                            h1t = hsb1.tile([128, D], BF16, tag="h1t")
                            nc.vector.tensor_copy(h1t[:], ps[:])
                            nc.sync.dma_start(
                                shard1[w * 128:(w + 1) * 128, :], h1t[:])
                            h1_tiles.append(h1t)
                    all_gather(shard1, table1, None, 0)
                    h2_tiles = layer(1, table1, h1_tiles, b1_sb, shard2,
                                     None, hsb2, make_next=True)
                all_gather(shard2, table2, None, 1)
                with tc.tile_pool(name="zsink", bufs=3) as zsink:
                    layer(2, table2, h2_tiles, b2_sb, shardz,
                          None, zsink, make_next=False)
            all_gather(shardz, tablez, None, 2)

            # decode
            with tc.tile_pool(name="didx", bufs=1) as didxp, \
                 tc.tile_pool(name="dM", bufs=1) as dMp, \
                 tc.tile_pool(name="dw", bufs=4) as dwp, \
                 tc.tile_pool(name="dout", bufs=1) as doutp:
                ds_sb = didxp.tile([128, TOT_DEC // 16], I16)
                nc.sync.dma_start(ds_sb[:], didx_s_in[:])
                dd_sb = didxp.tile([128, TOT_DEC // 16], I16)
                nc.sync.dma_start(dd_sb[:], didx_d_in[:])
                Ms = dMp.tile([128, TOT_DEC // 128, D], BF16, tag="Ms")
                Md = dMp.tile([128, TOT_DEC // 128, D], BF16, tag="Md")
                res = doutp.tile([128, TOT_DEC // 128], F32)
                NG_DEC = c.NCH * c.NCH
                for g in range(NG_DEC):
                    chs, chd = g // c.NCH, g % c.NCH
                    off16 = g * B_dec // 16
                    coff = g * B_dec // 128
                    ncols = B_dec // 128
                    emit_gather(
                        Ms[:, coff:coff + ncols, :],
                        tablez[chs * c.CHROWS:(chs + 1) * c.CHROWS, :],
                        ds_sb[:, off16:off16 + B_dec // 16], B_dec)
                    emit_gather(
                        Md[:, coff:coff + ncols, :],
                        tablez[chd * c.CHROWS:(chd + 1) * c.CHROWS, :],
                        dd_sb[:, off16:off16 + B_dec // 16], B_dec)
                for col in range(TOT_DEC // 128):
                    mm = dwp.tile([128, D], F32, tag="mm")
                    nc.vector.tensor_tensor(
                        mm[:], Ms[:, col, :], Md[:, col, :],
                        op=mybir.AluOpType.mult)
                    nc.vector.reduce_sum(res[:, col:col + 1], mm[:],
                                         axis=mybir.AxisListType.X)
                nc.sync.dma_start(dots_out[:], res[:])

    nc.compile()
    return nc


def assemble_output(cfg, meta, results):
    c = cfg
    slot2j = meta["slot2j"]
    out = np.zeros(c.EL, dtype=np.float32)
    for core in range(len(results)):
        d = np.asarray(results[core]["dots"], dtype=np.float32)
        flat = d.T.reshape(-1)             # slot i -> d[i%128, i//128]
        s2j = slot2j[core]
        valid = s2j >= 0
        out[s2j[valid]] = flat[valid]
    return out


def run_pipeline(x, edge_index, edge_label_index, W1, b1, W2, b2,
                 cfg=None, trace=False, tmpdir=None):
    cfg = cfg or DEFAULT
    in_maps, meta = host_prep(cfg, x, edge_index, edge_label_index,
                              W1, b1, W2, b2)
    nc = build_program(cfg, meta)
    res = run_bass_kernel_spmd(nc, in_maps, list(range(cfg.NC)),
                               trace=trace, tmpdir=tmpdir)
    return assemble_output(cfg, meta, res.results), res


def kernel(x, edge_index, edge_label_index, W1, b1, W2, b2):
    out, _ = run_pipeline(x, edge_index, edge_label_index, W1, b1, W2, b2)
    return out



# revision 25
# speedup vs baseline: 1.5155x; 1.5155x over previous
"""Trainium2 Bass kernel for nn_LinkPredictor (2-layer GCN + edge-dot decode).

Strategy (8 NeuronCores, SPMD), v2 "aggregate-then-transform":
  - GCN algebra: out[d] = relu(dinv[d] * (sum_{e:dst=d} g[src_e]) @ W + b)
    with g[n] = dinv[n] * z[n] and self-loops treated as ordinary edges.
    Aggregation happens in INPUT feature space (associativity), so the
    per-layer table holds g (bf16 rows) and the W matmul runs once per
    128-node window instead of once per node table entry.
  - Layer 1's table g0 = dinv * x is precomputed on host and staged to
    every core -> no first AllGather and no h1 precompute phase.
  - Nodes sharded: core c owns rows [c*12544, (c+1)*12544).  Edges assigned
    to the core owning their dst, grouped by (dst window of 128 nodes,
    src chunk) with a uniform slot budget B per group.  6 OVERLAPPING src
    chunks (reach 32768 for int16 idx) + greedy 2-3-choice balancing keep
    B at ~384.
  - Aggregation: PE matmul agg^T[inD,dst] += M_tile^T @ S01_tile where
    M_tile = gathered g rows (lhsT/weights) and S01 = one-hot slot->dst
    matrix (rhs).  S01 entries are pure 0/1 (no per-edge norm!), built
    32-96 tiles at a time with a single wide DVE is_equal over broadcast
    access patterns.
  - Epilogue per window: PSUM->SBUF cast on the (idle) Scalar engine,
    one PE matmul with W, DVE (mm*dinv)+bias, relu(+dinv scale for the
    g table) -> DMA to shard.
  - Inter-layer full-table exchange via AllGather into Shared DRAM.
  - Decode: gather z2[s], z2[d] per label edge, fused multiply+reduce.
"""
import contextlib
import math
import os
import numpy as np
import ml_dtypes

import concourse.bass as bass
import concourse.tile as tile
from concourse import bacc, mybir
from concourse.bass_utils import run_bass_kernel_spmd
from concourse.tile_rust import add_dep_helper

F32 = mybir.dt.float32
BF16 = mybir.dt.bfloat16
I16 = mybir.dt.int16
BF = ml_dtypes.bfloat16

CH_REACH = 32768            # int16 index reach for dma_gather

# dev bisect switches (default = full-fat kernel)
S_WIDE = os.environ.get("S_WIDE", "1") == "1"
STT = os.environ.get("STT", "1") == "1"
TTR = os.environ.get("TTR", "0") == "1"   # InstTensorTensorReduce crashes HW
SCOPY = os.environ.get("SCOPY", "0") == "1"  # ACTIVATE-Copy-from-PSUM crashes HW
T0CHUNK = os.environ.get("T0CHUNK", "1") == "1"
IDENT = os.environ.get("IDENT", "1") == "1"


class Cfg:
    def __init__(self, N=100000, E=1600000, EL=100000, D=128, ncores=8, nw=98,
                 wb=4):
        self.N, self.E, self.EL, self.D, self.NC = N, E, EL, D, ncores
        self.NW = nw                      # windows (128 nodes each) per core
        self.SHARD = nw * 128             # nodes per core (padded)
        self.NP = self.SHARD * ncores     # padded node count
        assert self.NP >= N
        # overlapping source chunks (each covers CH_REACH rows)
        self.BASES = [0, 13440, 26880, 40320, 53760, self.NP - CH_REACH]
        assert all(b2 - b1 < CH_REACH for b1, b2 in
                   zip(self.BASES, self.BASES[1:]))
        self.NCH = len(self.BASES)
        self.WB = wb                      # windows per gather/aggregate batch
        self.NBATCH = math.ceil(nw / wb)


DEFAULT = Cfg()


def _wrap_idxs(idx):
    """[n] ints -> [128, n//16] int16 wrapped in 16 partitions, replicated 8x."""
    n = len(idx)
    assert n % 16 == 0
    w = np.asarray(idx, dtype=np.int16).reshape(n // 16, 16).T
    return np.ascontiguousarray(np.tile(w, (8, 1)))


def _balance_chunks(c, s, w, nw):
    """Greedily assign each edge to an eligible src chunk, balancing
    (window, chunk) group sizes.  Returns (ch_of, counts)."""
    bases = np.asarray(c.BASES)
    hi = np.searchsorted(bases, s, side="right") - 1
    lo = np.searchsorted(bases, s - (CH_REACH - 1), side="left")
    counts = np.zeros((nw, c.NCH), dtype=np.int64)
    ch_of = np.empty(len(s), dtype=np.int64)
    # least-flexible edges first so forced chunks fill before shared ones
    order = np.lexsort((hi - lo, w))
    wl, lol, hil = w.tolist(), lo.tolist(), hi.tolist()
    for e in order.tolist():
        we, l, h = wl[e], lol[e], hil[e]
        row = counts[we]
        best = l
        for ch in range(l + 1, h + 1):
            if row[ch] < row[best]:
                best = ch
        ch_of[e] = best
        row[best] += 1
    return ch_of, counts


def host_prep(cfg, x, edge_index, edge_label_index, W1, b1, W2, b2):
    """All host-side sharding/layout. Returns (per-core input maps, meta)."""
    c = cfg
    # --- degrees / normalization (self-loop included, as in PyG GCNConv)
    src = np.asarray(edge_index[0], dtype=np.int64)
    dst = np.asarray(edge_index[1], dtype=np.int64)
    deg = np.bincount(dst, minlength=c.N).astype(np.float64) + 1.0
    dinv = 1.0 / np.sqrt(deg)                           # [N]

    # --- node permutation: serpentine-deal degree-sorted nodes across all
    # core*window bins so every 128-node window gets ~equal in-edge count
    # (tightens the per-(window,chunk) slot budget B).
    NWIN = c.NC * c.NW
    degp = np.zeros(c.NP)
    degp[:c.N] = deg
    order = np.argsort(-degp, kind="stable")            # node ids, deg desc
    node_at = np.empty(c.NP, dtype=np.int64)            # position -> node
    for r in range(128):
        blk = order[r * NWIN:(r + 1) * NWIN]
        wins = np.arange(NWIN) if r % 2 == 0 else np.arange(NWIN - 1, -1, -1)
        node_at[wins * 128 + r] = blk
    pos_of = np.empty(c.NP, dtype=np.int64)             # node -> position
    pos_of[node_at] = np.arange(c.NP)

    dinv_p = np.zeros(c.NP, dtype=np.float64)
    dinv_p[pos_of[:c.N]] = dinv                         # pad positions -> 0

    # self-loops are NOT edges here: they are added on-device as one
    # identity-rhs matmul per window (agg += G_own^T @ I), so they cost no
    # gather slots and no chunk-eligibility pressure.
    s_all = pos_of[src]
    d_all = pos_of[dst]

    # --- per-core edge grouping with balanced overlapping chunks
    core_of = d_all // c.SHARD
    B = 0
    per_core = []
    for core in range(c.NC):
        m = core_of == core
        s_c, d_c = s_all[m], d_all[m]
        w_c = (d_c - core * c.SHARD) // 128
        dl_c = (d_c - core * c.SHARD) % 128
        ch_c, counts = _balance_chunks(c, s_c, w_c, c.NW)
        per_core.append((s_c, w_c, dl_c, ch_c, counts))
        B = max(B, int(counts.max()))
    B = 128 * math.ceil(max(B, 1) / 128)
    TPG = B // 128
    TOT = c.NW * c.NCH * B
    TOT_TILES = TOT // 128

    bases = np.asarray(c.BASES)
    idx_arr = np.zeros((c.NC, TOT), dtype=np.int64)     # pad -> row 0 of chunk
    dstloc_arr = np.full((c.NC, TOT), -1.0, dtype=np.float32)
    for core in range(c.NC):
        s_c, w_c, dl_c, ch_c, counts = per_core[core]
        key = (w_c * c.NCH + ch_c)
        order = np.argsort(key, kind="stable")
        ngroups = c.NW * c.NCH
        cnt = np.bincount(key, minlength=ngroups)
        starts = np.zeros(ngroups + 1, dtype=np.int64)
        np.cumsum(cnt, out=starts[1:])
        # slot order: batch -> chunk -> window-in-batch -> slots
        pos = 0
        for b in range(c.NBATCH):
            wlo, whi = b * c.WB, min((b + 1) * c.WB, c.NW)
            for ch in range(c.NCH):
                for w in range(wlo, whi):
                    g = w * c.NCH + ch
                    eids = order[starts[g]:starts[g + 1]]
                    n = len(eids)
                    assert n <= B
                    idx_arr[core, pos:pos + n] = s_c[eids] - bases[ch]
                    dstloc_arr[core, pos:pos + n] = dl_c[eids]
                    pos += B
        assert pos == TOT

    # --- decode: label edge j -> core j // ELC; groups by (chunk(d), chunk(s))
    assert c.EL % c.NC == 0
    ELC = c.EL // c.NC
    ls = pos_of[np.asarray(edge_label_index[0], dtype=np.int64)]
    ld = pos_of[np.asarray(edge_label_index[1], dtype=np.int64)]
    his = np.searchsorted(bases, ls, side="right") - 1
    los = np.searchsorted(bases, ls - (CH_REACH - 1), side="left")
    hid = np.searchsorted(bases, ld, side="right") - 1
    lod = np.searchsorted(bases, ld - (CH_REACH - 1), side="left")
    NG_DEC = c.NCH * c.NCH
    grp = np.empty((c.NC, ELC), dtype=np.int64)
    B_dec = 0
    for core in range(c.NC):
        cnts = np.zeros(NG_DEC, dtype=np.int64)
        jlo = core * ELC
        for j in range(jlo, jlo + ELC):
            best, bestv = -1, 1 << 60
            for chd in range(lod[j], hid[j] + 1):
                for chs in range(los[j], his[j] + 1):
                    g = chd * c.NCH + chs
                    if cnts[g] < bestv:
                        best, bestv = g, cnts[g]
            grp[core, j - jlo] = best
            cnts[best] += 1
        B_dec = max(B_dec, int(cnts.max()))
    B_dec = 128 * math.ceil(max(B_dec, 1) / 128)
    TOT_DEC = NG_DEC * B_dec
    idx_s = np.zeros((c.NC, TOT_DEC), dtype=np.int64)
    idx_d = np.zeros((c.NC, TOT_DEC), dtype=np.int64)
    slot2j = np.full((c.NC, TOT_DEC), -1, dtype=np.int64)
    for core in range(c.NC):
        jlo = core * ELC
        kk = grp[core]
        o = np.argsort(kk, kind="stable")
        cnt = np.bincount(kk, minlength=NG_DEC)
        st = np.zeros(NG_DEC + 1, dtype=np.int64)
        np.cumsum(cnt, out=st[1:])
        for g in range(NG_DEC):
            js = o[st[g]:st[g + 1]] + jlo
            n = len(js)
            pos = g * B_dec
            chd, chs = g // c.NCH, g % c.NCH
            idx_s[core, pos:pos + n] = ls[js] - bases[chs]
            idx_d[core, pos:pos + n] = ld[js] - bases[chd]
            slot2j[core, pos:pos + n] = js

    # --- staged tensors (g0 rows live at permuted positions)
    xp = np.zeros((c.NP, c.D), dtype=np.float64)
    xp[pos_of[:c.N]] = np.asarray(x, dtype=np.float64)
    g0 = np.ascontiguousarray((xp * dinv_p[:, None]).astype(np.float32)).astype(BF)
    dinv_f = dinv_p.astype(np.float32)
    in_maps = []
    for core in range(c.NC):
        sl = slice(core * c.SHARD, (core + 1) * c.SHARD)
        in_maps.append({
            "g0tab": g0,
            "g0own": np.ascontiguousarray(g0[sl]),
            "W1": np.asarray(W1, dtype=np.float32).astype(BF),
            "W2": np.asarray(W2, dtype=np.float32).astype(BF),
            "b1r": np.tile(np.asarray(b1, np.float32)[None, :], (128, 1)),
            "b2r": np.tile(np.asarray(b2, np.float32)[None, :], (128, 1)),
            "dinvw": np.ascontiguousarray(dinv_f[sl].reshape(c.NW, 128).T),
            "gidx": _wrap_idxs(idx_arr[core]),
            "dstloc": np.ascontiguousarray(
                dstloc_arr[core].reshape(TOT_TILES, 128).T).astype(BF),
            "dstloc32": np.ascontiguousarray(
                dstloc_arr[core].reshape(TOT_TILES, 128).T),
            "didx_s": _wrap_idxs(idx_s[core]),
            "didx_d": _wrap_idxs(idx_d[core]),
        })
    meta = dict(B=B, TPG=TPG, TOT=TOT, TOT_TILES=TOT_TILES,
                B_dec=B_dec, TOT_DEC=TOT_DEC, slot2j=slot2j)
    return in_maps, meta


def build_program(cfg, meta, num_cores=None):
    c = cfg
    NCores = num_cores or c.NC
    B, TPG, TOT, TOT_TILES = meta["B"], meta["TPG"], meta["TOT"], meta["TOT_TILES"]
    B_dec, TOT_DEC = meta["B_dec"], meta["TOT_DEC"]
    D = c.D

    nc = bacc.Bacc("TRN2", target_bir_lowering=False, debug=False,
                   num_devices=NCores, num_swdge_queues=4)
    NQ = 4

    g0_in = nc.dram_tensor("g0tab", [c.NP, D], BF16, kind="ExternalInput")
    g0own_in = nc.dram_tensor("g0own", [c.SHARD, D], BF16, kind="ExternalInput")
    W1_in = nc.dram_tensor("W1", [D, D], BF16, kind="ExternalInput")
    W2_in = nc.dram_tensor("W2", [D, D], BF16, kind="ExternalInput")
    b1_in = nc.dram_tensor("b1r", [128, D], F32, kind="ExternalInput")
    b2_in = nc.dram_tensor("b2r", [128, D], F32, kind="ExternalInput")
    dinvw_in = nc.dram_tensor("dinvw", [128, c.NW], F32, kind="ExternalInput")
    gidx_in = nc.dram_tensor("gidx", [128, TOT // 16], I16, kind="ExternalInput")
    dstloc_in = nc.dram_tensor("dstloc", [128, TOT_TILES], BF16, kind="ExternalInput")
    dstloc32_in = nc.dram_tensor("dstloc32", [128, TOT_TILES], F32, kind="ExternalInput")
    didx_s_in = nc.dram_tensor("didx_s", [128, TOT_DEC // 16], I16, kind="ExternalInput")
    didx_d_in = nc.dram_tensor("didx_d", [128, TOT_DEC // 16], I16, kind="ExternalInput")
    dots_out = nc.dram_tensor("dots", [128, TOT_DEC // 128], F32, kind="ExternalOutput")

    gst = {"count": 0, "prev": None}

    def emit_gather(out_ap, in_ap, idx_ap, n_idx, extra_deps=()):
        q = gst["count"] % NQ
        inst = nc.gpsimd.dma_gather(out_ap, in_ap, idx_ap, n_idx, n_idx, D,
                                    queue_num=q, single_packet=False)
        if gst["prev"] is not None:
            add_dep_helper(inst.ins, gst["prev"].ins, sync=False,
                           reason="pin swdge queue order")
        for dep in extra_deps:
            add_dep_helper(inst.ins, dep.ins, sync=True,
                           reason="gather reads DRAM written by this DMA")
        gst["prev"] = inst
        gst["count"] += 1
        return inst

    shard1 = nc.dram_tensor("shard1", [c.SHARD, D], BF16)
    shardz = nc.dram_tensor("shardz", [c.SHARD, D], BF16)
    table0 = nc.dram_tensor("table0", [c.NP, D], BF16)
    table1 = nc.dram_tensor("table1", [c.NP, D], BF16)
    tablez = nc.dram_tensor("tablez", [c.NP, D], BF16)

    iota_dram = nc.inline_tensor(
        np.tile(np.arange(128, dtype=np.float32), (128, 1)).astype(BF), "iota_c")
    ident_dram = nc.inline_tensor(np.eye(128, dtype=np.float32).astype(BF), "ident_c")

    cc_sem = nc.alloc_semaphore("cc_sem")
    core_ids = list(range(NCores))

    with tile.TileContext(nc) as tc:
        with contextlib.ExitStack() as es:
            const = es.enter_context(tc.tile_pool(name="const", bufs=1))
            meta_p = es.enter_context(tc.tile_pool(name="meta", bufs=1))

            w1_sb = const.tile([D, D], BF16); nc.sync.dma_start(w1_sb[:], W1_in[:])
            w2_sb = const.tile([D, D], BF16); nc.sync.dma_start(w2_sb[:], W2_in[:])
            b1_sb = const.tile([128, D], F32); nc.sync.dma_start(b1_sb[:], b1_in[:])
            b2_sb = const.tile([128, D], F32); nc.sync.dma_start(b2_sb[:], b2_in[:])
            dinvw_sb = const.tile([128, c.NW], F32)
            nc.sync.dma_start(dinvw_sb[:], dinvw_in[:])
            iota_sb = const.tile([128, 128], BF16)
            nc.sync.dma_start(iota_sb[:], iota_dram[:])
            ident_sb = const.tile([128, 128], BF16)
            nc.sync.dma_start(ident_sb[:], ident_dram[:])
            zeros_sb = const.tile([128, 128], F32)
            nc.vector.memset(zeros_sb[:], 0.0)
            # gathers must source an Internal DRAM tensor: copy staged g0 in
            t0_copies = []
            if T0CHUNK:
                step = c.NP // 8
                for i in range(8):
                    t0_copies.append(nc.sync.dma_start(
                        table0[i * step:(i + 1) * step, :],
                        g0_in[i * step:(i + 1) * step, :]))
            else:
                t0_copies.append(nc.sync.dma_start(table0[:], g0_in[:]))
            gidx_sb = meta_p.tile([128, TOT // 16], I16)
            nc.sync.dma_start(gidx_sb[:], gidx_in[:])
            dstloc_sb = meta_p.tile([128, TOT_TILES], BF16)
            nc.sync.dma_start(dstloc_sb[:], dstloc_in[:])
            dstloc32_sb = meta_p.tile([128, TOT_TILES], F32)
            nc.sync.dma_start(dstloc32_sb[:], dstloc32_in[:])

            def all_gather(shard, table_out, n_before):
                tc.strict_bb_all_engine_barrier()
                with tc.tile_critical():
                    nc.gpsimd.collective_compute(
                        "AllGather", mybir.AluOpType.bypass,
                        replica_groups=[core_ids],
                        ins=[shard[:]], outs=[table_out[:]],
                    ).then_inc(cc_sem)
                    nc.gpsimd.wait_ge(cc_sem, n_before + 1)
                tc.strict_bb_all_engine_barrier()

            def layer(lid, table, w_sb, bias_sb, outer_dinv, shard_next,
                      own_tiles, own_src, out_pool, first_deps=()):
                """own_tiles: list of 98 SBUF [128, D] bf16 tiles holding this
                core's own g rows per window (self-loop term); if None, they
                are DMA-loaded from own_src ([SHARD, D] per-core tensor).
                Returns the produced zt tiles (allocated from out_pool)."""
                out_tiles = []
                with tc.tile_pool(name=f"M{lid}", bufs=2) as Mp, \
                     tc.tile_pool(name=f"S{lid}", bufs=2) as Sp, \
                     tc.tile_pool(name=f"own{lid}", bufs=2 * c.WB) as ownp, \
                     tc.tile_pool(name=f"ag{lid}", bufs=4, space="PSUM") as agp, \
                     tc.tile_pool(name=f"mm{lid}", bufs=2, space="PSUM") as mmp, \
                     tc.tile_pool(name=f"ep{lid}", bufs=3) as epp:
                    for b in range(c.NBATCH):
                        wlo = b * c.WB
                        whi = min(wlo + c.WB, c.NW)
                        nwb = whi - wlo
                        cols_per_ch = nwb * TPG
                        ntiles = c.NCH * cols_per_ch
                        slot_base = wlo * c.NCH * B
                        tile_base = slot_base // 128
                        Mt = Mp.tile([128, ntiles, D], BF16, tag="M")
                        for ch in range(c.NCH):
                            n_idx = nwb * B
                            off16 = (slot_base + ch * n_idx) // 16
                            emit_gather(
                                Mt[:, ch * cols_per_ch:(ch + 1) * cols_per_ch, :],
                                table[c.BASES[ch]:c.BASES[ch] + CH_REACH, :],
                                gidx_sb[:, off16:off16 + n_idx // 16],
                                n_idx,
                                extra_deps=(first_deps if b == 0 and ch == 0
                                            else ()))
                        if own_tiles is None:
                            ownts = []
                            for wi in range(nwb):
                                w = wlo + wi
                                ot = ownp.tile([128, D], BF16, tag="own")
                                nc.sync.dma_start(
                                    ot[:], own_src[w * 128:(w + 1) * 128, :])
                                ownts.append(ot)
                        St = Sp.tile([128, ntiles, 128], BF16, tag="S")
                        if S_WIDE:
                            nc.vector.tensor_tensor(
                                St[:],
                                iota_sb[:, None, :].broadcast_to([128, ntiles, 128]),
                                dstloc_sb[:, tile_base:tile_base + ntiles, None]
                                    .broadcast_to([128, ntiles, 128]),
                                op=mybir.AluOpType.is_equal)
                        else:
                            for k in range(ntiles):
                                nc.vector.tensor_scalar(
                                    St[:, k, :], iota_sb[:],
                                    dstloc32_sb[:, tile_base + k:tile_base + k + 1],
                                    None, mybir.AluOpType.is_equal)
                        for wi in range(nwb):
                            w = wlo + wi
                            agg = agp.tile([128, D], F32, tag="agg")
                            nmm = c.NCH * TPG + 1
                            k = 0
                            for ch in range(c.NCH):
                                for t in range(TPG):
                                    mcol = (ch * nwb + wi) * TPG + t
                                    nc.tensor.matmul(
                                        agg[:], lhsT=Mt[:, mcol, :],
                                        rhs=St[:, mcol, :],
                                        start=(k == 0), stop=False)
                                    k += 1
                            self_lhs = (own_tiles[w][:] if own_tiles is not None
                                        else ownts[wi][:])
                            if IDENT:
                                nc.tensor.matmul(agg[:], lhsT=self_lhs,
                                                 rhs=ident_sb[:],
                                                 start=False, stop=True)
                            else:
                                nc.tensor.matmul(agg[:], lhsT=self_lhs,
                                                 rhs=ident_sb[:],
                                                 start=False, stop=True)
                            aggT_sb = epp.tile([128, D], BF16, tag="aggT")
                            if SCOPY:
                                nc.scalar.copy(aggT_sb[:], agg[:])
                            else:
                                nc.vector.tensor_copy(aggT_sb[:], agg[:])
                            mm = mmp.tile([128, D], F32, tag="mm")
                            nc.tensor.matmul(mm[:], lhsT=aggT_sb[:], rhs=w_sb[:],
                                             start=True, stop=True)
                            v = epp.tile([128, D], F32, tag="v")
                            if STT:
                                nc.vector.scalar_tensor_tensor(
                                    v[:], mm[:], dinvw_sb[:, w:w + 1], bias_sb[:],
                                    op0=mybir.AluOpType.mult,
                                    op1=mybir.AluOpType.add)
                            else:
                                nc.vector.tensor_scalar(
                                    v[:], mm[:], dinvw_sb[:, w:w + 1], None,
                                    mybir.AluOpType.mult)
                                nc.vector.tensor_tensor(
                                    v[:], v[:], bias_sb[:],
                                    op=mybir.AluOpType.add)
                            zt = out_pool.tile([128, D], BF16, tag=f"z{lid}")
                            if outer_dinv:
                                if STT:
                                    nc.vector.scalar_tensor_tensor(
                                        zt[:], v[:], dinvw_sb[:, w:w + 1],
                                        zeros_sb[:],
                                        op0=mybir.AluOpType.mult,
                                        op1=mybir.AluOpType.max)
                                else:
                                    rt = epp.tile([128, D], F32, tag="rt")
                                    nc.scalar.activation(
                                        rt[:], v[:],
                                        mybir.ActivationFunctionType.Relu)
                                    nc.vector.tensor_scalar(
                                        zt[:], rt[:], dinvw_sb[:, w:w + 1], None,
                                        mybir.AluOpType.mult)
                            else:
                                nc.scalar.activation(
                                    zt[:], v[:], mybir.ActivationFunctionType.Relu)
                            nc.sync.dma_start(
                                shard_next[w * 128:(w + 1) * 128, :], zt[:])
                            out_tiles.append(zt)
                return out_tiles

            with tc.tile_pool(name="g1keep", bufs=c.NW) as g1p:
                with tc.tile_pool(name="zsink", bufs=3) as zsink:
                    g1_tiles = layer(1, table0, w1_sb, b1_sb, True, shard1,
                                     None, g0own_in, g1p, first_deps=t0_copies)
                    all_gather(shard1, table1, 0)
                    layer(2, table1, w2_sb, b2_sb, False, shardz,
                          g1_tiles, None, zsink)
            all_gather(shardz, tablez, 1)

            # decode
            DCOLS = TOT_DEC // 128
            with tc.tile_pool(name="didx", bufs=1) as didxp, \
                 tc.tile_pool(name="dM", bufs=1) as dMp, \
                 tc.tile_pool(name="dw", bufs=4) as dwp, \
                 tc.tile_pool(name="dout", bufs=1) as doutp:
                ds_sb = didxp.tile([128, TOT_DEC // 16], I16)
                nc.sync.dma_start(ds_sb[:], didx_s_in[:])
                dd_sb = didxp.tile([128, TOT_DEC // 16], I16)
                nc.sync.dma_start(dd_sb[:], didx_d_in[:])
                Ms = dMp.tile([128, DCOLS, D], BF16, tag="Ms")
                Md = dMp.tile([128, DCOLS, D], BF16, tag="Md")
                res = doutp.tile([128, DCOLS], F32)
                # d-side: one gather per chd (contiguous NCH*B_dec slots)
                for chd in range(c.NCH):
                    n_idx = c.NCH * B_dec
                    off16 = chd * n_idx // 16
                    coff = chd * n_idx // 128
                    emit_gather(
                        Md[:, coff:coff + n_idx // 128, :],
                        tablez[c.BASES[chd]:c.BASES[chd] + CH_REACH, :],
                        dd_sb[:, off16:off16 + n_idx // 16], n_idx)
                # s-side: one gather per (chd, chs) group
                for g in range(c.NCH * c.NCH):
                    chs = g % c.NCH
                    off16 = g * B_dec // 16
                    coff = g * B_dec // 128
                    emit_gather(
                        Ms[:, coff:coff + B_dec // 128, :],
                        tablez[c.BASES[chs]:c.BASES[chs] + CH_REACH, :],
                        ds_sb[:, off16:off16 + B_dec // 16], B_dec)
                for col in range(DCOLS):
                    sc = dwp.tile([128, D], F32, tag="sc")
                    if TTR:
                        nc.vector.tensor_tensor_reduce(
                            out=sc[:], in0=Ms[:, col, :], in1=Md[:, col, :],
                            scale=1.0, scalar=0.0,
                            op0=mybir.AluOpType.mult, op1=mybir.AluOpType.add,
                            accum_out=res[:, col:col + 1])
                    else:
                        nc.vector.tensor_tensor(
                            sc[:], Ms[:, col, :], Md[:, col, :],
                            op=mybir.AluOpType.mult)
                        nc.vector.reduce_sum(res[:, col:col + 1], sc[:],
                                             axis=mybir.AxisListType.X)
                nc.sync.dma_start(dots_out[:], res[:])

    nc.compile()
    return nc


def assemble_output(cfg, meta, results):
    c = cfg
    slot2j = meta["slot2j"]
    out = np.zeros(c.EL, dtype=np.float32)
    for core in range(len(results)):
        d = np.asarray(results[core]["dots"], dtype=np.float32)
        flat = d.T.reshape(-1)             # slot i -> d[i%128, i//128]
        s2j = slot2j[core]
        valid = s2j >= 0
        out[s2j[valid]] = flat[valid]
    return out


def run_pipeline(x, edge_index, edge_label_index, W1, b1, W2, b2,
                 cfg=None, trace=False, tmpdir=None):
    cfg = cfg or DEFAULT
    in_maps, meta = host_prep(cfg, x, edge_index, edge_label_index,
                              W1, b1, W2, b2)
    nc = build_program(cfg, meta)
    res = run_bass_kernel_spmd(nc, in_maps, list(range(cfg.NC)),
                               trace=trace, tmpdir=tmpdir)
    return assemble_output(cfg, meta, res.results), res


def kernel(x, edge_index, edge_label_index, W1, b1, W2, b2):
    out, _ = run_pipeline(x, edge_index, edge_label_index, W1, b1, W2, b2)
    return out


# revision 31
# speedup vs baseline: 1.6962x; 1.1193x over previous
"""Trainium2 Bass kernel for nn_LinkPredictor (2-layer GCN + edge-dot decode).

Strategy (8 NeuronCores, SPMD), v2 "aggregate-then-transform":
  - GCN algebra: out[d] = relu(dinv[d] * (sum_{e:dst=d} g[src_e]) @ W + b)
    with g[n] = dinv[n] * z[n] and self-loops treated as ordinary edges.
    Aggregation happens in INPUT feature space (associativity), so the
    per-layer table holds g (bf16 rows) and the W matmul runs once per
    128-node window instead of once per node table entry.
  - Layer 1's table g0 = dinv * x is precomputed on host and staged to
    every core -> no first AllGather and no h1 precompute phase.
  - Nodes sharded: core c owns rows [c*12544, (c+1)*12544).  Edges assigned
    to the core owning their dst, grouped by (dst window of 128 nodes,
    src chunk) with a uniform slot budget B per group.  6 OVERLAPPING src
    chunks (reach 32768 for int16 idx) + greedy 2-3-choice balancing keep
    B at ~384.
  - Aggregation: PE matmul agg^T[inD,dst] += M_tile^T @ S01_tile where
    M_tile = gathered g rows (lhsT/weights) and S01 = one-hot slot->dst
    matrix (rhs).  S01 entries are pure 0/1 (no per-edge norm!), built
    32-96 tiles at a time with a single wide DVE is_equal over broadcast
    access patterns.
  - Epilogue per window: PSUM->SBUF cast on the (idle) Scalar engine,
    one PE matmul with W, DVE (mm*dinv)+bias, relu(+dinv scale for the
    g table) -> DMA to shard.
  - Inter-layer full-table exchange via AllGather into Shared DRAM.
  - Decode: gather z2[s], z2[d] per label edge, fused multiply+reduce.
"""
import contextlib
import math
import os
import numpy as np
import ml_dtypes

import concourse.bass as bass
import concourse.tile as tile
from concourse import bacc, mybir
from concourse.bass_utils import run_bass_kernel_spmd
from concourse.tile_rust import add_dep_helper

F32 = mybir.dt.float32
BF16 = mybir.dt.bfloat16
I16 = mybir.dt.int16
BF = ml_dtypes.bfloat16

CH_REACH = 32768            # int16 index reach for dma_gather

# dev bisect switches (default = full-fat kernel)
S_WIDE = os.environ.get("S_WIDE", "1") == "1"
STT = os.environ.get("STT", "1") == "1"
TTR = os.environ.get("TTR", "0") == "1"   # InstTensorTensorReduce crashes HW
SCOPY = os.environ.get("SCOPY", "0") == "1"  # ACTIVATE-Copy-from-PSUM crashes HW
T0CHUNK = os.environ.get("T0CHUNK", "1") == "1"
IDENT = os.environ.get("IDENT", "1") == "1"
T0DIRECT = os.environ.get("T0DIRECT", "1") == "1"  # gather L1 straight from staged input
AGCHUNK = int(os.environ.get("AGCHUNK", "0"))      # chunks per AllGather (0 = barrier AG)


class Cfg:
    def __init__(self, N=100000, E=1600000, EL=100000, D=128, ncores=8, nw=98,
                 wb=4):
        self.N, self.E, self.EL, self.D, self.NC = N, E, EL, D, ncores
        self.NW = nw                      # windows (128 nodes each) per core
        self.SHARD = nw * 128             # nodes per core (padded)
        self.NP = self.SHARD * ncores     # padded node count
        assert self.NP >= N
        # overlapping source chunks (each covers CH_REACH rows)
        self.BASES = [0, 13440, 26880, 40320, 53760, self.NP - CH_REACH]
        assert all(b2 - b1 < CH_REACH for b1, b2 in
                   zip(self.BASES, self.BASES[1:]))
        self.NCH = len(self.BASES)
        self.WB = wb                      # windows per gather/aggregate batch
        self.NBATCH = math.ceil(nw / wb)


DEFAULT = Cfg()


def _wrap_idxs(idx):
    """[n] ints -> [128, n//16] int16 wrapped in 16 partitions, replicated 8x."""
    n = len(idx)
    assert n % 16 == 0
    w = np.asarray(idx, dtype=np.int16).reshape(n // 16, 16).T
    return np.ascontiguousarray(np.tile(w, (8, 1)))


def _balance_chunks(c, s, w, nw):
    """Greedily assign each edge to an eligible src chunk, balancing
    (window, chunk) group sizes.  Returns (ch_of, counts)."""
    bases = np.asarray(c.BASES)
    hi = np.searchsorted(bases, s, side="right") - 1
    lo = np.searchsorted(bases, s - (CH_REACH - 1), side="left")
    counts = np.zeros((nw, c.NCH), dtype=np.int64)
    ch_of = np.empty(len(s), dtype=np.int64)
    # least-flexible edges first so forced chunks fill before shared ones
    order = np.lexsort((hi - lo, w))
    wl, lol, hil = w.tolist(), lo.tolist(), hi.tolist()
    for e in order.tolist():
        we, l, h = wl[e], lol[e], hil[e]
        row = counts[we]
        best = l
        for ch in range(l + 1, h + 1):
            if row[ch] < row[best]:
                best = ch
        ch_of[e] = best
        row[best] += 1
    return ch_of, counts


def host_prep(cfg, x, edge_index, edge_label_index, W1, b1, W2, b2):
    """All host-side sharding/layout. Returns (per-core input maps, meta)."""
    c = cfg
    # --- degrees / normalization (self-loop included, as in PyG GCNConv)
    src = np.asarray(edge_index[0], dtype=np.int64)
    dst = np.asarray(edge_index[1], dtype=np.int64)
    deg = np.bincount(dst, minlength=c.N).astype(np.float64) + 1.0
    dinv = 1.0 / np.sqrt(deg)                           # [N]

    # --- node permutation: serpentine-deal degree-sorted nodes across all
    # core*window bins so every 128-node window gets ~equal in-edge count
    # (tightens the per-(window,chunk) slot budget B).
    NWIN = c.NC * c.NW
    degp = np.zeros(c.NP)
    degp[:c.N] = deg
    order = np.argsort(-degp, kind="stable")            # node ids, deg desc
    node_at = np.empty(c.NP, dtype=np.int64)            # position -> node
    for r in range(128):
        blk = order[r * NWIN:(r + 1) * NWIN]
        wins = np.arange(NWIN) if r % 2 == 0 else np.arange(NWIN - 1, -1, -1)
        node_at[wins * 128 + r] = blk
    pos_of = np.empty(c.NP, dtype=np.int64)             # node -> position
    pos_of[node_at] = np.arange(c.NP)

    dinv_p = np.zeros(c.NP, dtype=np.float64)
    dinv_p[pos_of[:c.N]] = dinv                         # pad positions -> 0

    # self-loops are NOT edges here: they are added on-device as one
    # identity-rhs matmul per window (agg += G_own^T @ I), so they cost no
    # gather slots and no chunk-eligibility pressure.
    s_all = pos_of[src]
    d_all = pos_of[dst]

    # --- per-core edge grouping with balanced overlapping chunks
    core_of = d_all // c.SHARD
    B = 0
    per_core = []
    for core in range(c.NC):
        m = core_of == core
        s_c, d_c = s_all[m], d_all[m]
        w_c = (d_c - core * c.SHARD) // 128
        dl_c = (d_c - core * c.SHARD) % 128
        ch_c, counts = _balance_chunks(c, s_c, w_c, c.NW)
        per_core.append((s_c, w_c, dl_c, ch_c, counts))
        B = max(B, int(counts.max()))
    B = 128 * math.ceil(max(B, 1) / 128)
    TPG = B // 128
    TOT = c.NW * c.NCH * B
    TOT_TILES = TOT // 128

    bases = np.asarray(c.BASES)
    idx_arr = np.zeros((c.NC, TOT), dtype=np.int64)     # pad -> row 0 of chunk
    dstloc_arr = np.full((c.NC, TOT), -1.0, dtype=np.float32)
    for core in range(c.NC):
        s_c, w_c, dl_c, ch_c, counts = per_core[core]
        key = (w_c * c.NCH + ch_c)
        order = np.argsort(key, kind="stable")
        ngroups = c.NW * c.NCH
        cnt = np.bincount(key, minlength=ngroups)
        starts = np.zeros(ngroups + 1, dtype=np.int64)
        np.cumsum(cnt, out=starts[1:])
        # slot order: batch -> chunk -> window-in-batch -> slots
        pos = 0
        for b in range(c.NBATCH):
            wlo, whi = b * c.WB, min((b + 1) * c.WB, c.NW)
            for ch in range(c.NCH):
                for w in range(wlo, whi):
                    g = w * c.NCH + ch
                    eids = order[starts[g]:starts[g + 1]]
                    n = len(eids)
                    assert n <= B
                    idx_arr[core, pos:pos + n] = s_c[eids] - bases[ch]
                    dstloc_arr[core, pos:pos + n] = dl_c[eids]
                    pos += B
        assert pos == TOT

    # --- decode: label edge j -> core j // ELC; groups by (chunk(d), chunk(s))
    assert c.EL % c.NC == 0
    ELC = c.EL // c.NC
    ls = pos_of[np.asarray(edge_label_index[0], dtype=np.int64)]
    ld = pos_of[np.asarray(edge_label_index[1], dtype=np.int64)]
    his = np.searchsorted(bases, ls, side="right") - 1
    los = np.searchsorted(bases, ls - (CH_REACH - 1), side="left")
    hid = np.searchsorted(bases, ld, side="right") - 1
    lod = np.searchsorted(bases, ld - (CH_REACH - 1), side="left")
    NG_DEC = c.NCH * c.NCH
    grp = np.empty((c.NC, ELC), dtype=np.int64)
    B_dec = 0
    for core in range(c.NC):
        cnts = np.zeros(NG_DEC, dtype=np.int64)
        jlo = core * ELC
        for j in range(jlo, jlo + ELC):
            best, bestv = -1, 1 << 60
            for chd in range(lod[j], hid[j] + 1):
                for chs in range(los[j], his[j] + 1):
                    g = chd * c.NCH + chs
                    if cnts[g] < bestv:
                        best, bestv = g, cnts[g]
            grp[core, j - jlo] = best
            cnts[best] += 1
        B_dec = max(B_dec, int(cnts.max()))
    B_dec = 128 * math.ceil(max(B_dec, 1) / 128)
    TOT_DEC = NG_DEC * B_dec
    idx_s = np.zeros((c.NC, TOT_DEC), dtype=np.int64)
    idx_d = np.zeros((c.NC, TOT_DEC), dtype=np.int64)
    slot2j = np.full((c.NC, TOT_DEC), -1, dtype=np.int64)
    for core in range(c.NC):
        jlo = core * ELC
        kk = grp[core]
        o = np.argsort(kk, kind="stable")
        cnt = np.bincount(kk, minlength=NG_DEC)
        st = np.zeros(NG_DEC + 1, dtype=np.int64)
        np.cumsum(cnt, out=st[1:])
        for g in range(NG_DEC):
            js = o[st[g]:st[g + 1]] + jlo
            n = len(js)
            pos = g * B_dec
            chd, chs = g // c.NCH, g % c.NCH
            idx_s[core, pos:pos + n] = ls[js] - bases[chs]
            idx_d[core, pos:pos + n] = ld[js] - bases[chd]
            slot2j[core, pos:pos + n] = js

    # --- staged tensors (g0 rows live at permuted positions)
    xp = np.zeros((c.NP, c.D), dtype=np.float64)
    xp[pos_of[:c.N]] = np.asarray(x, dtype=np.float64)
    g0 = np.ascontiguousarray((xp * dinv_p[:, None]).astype(np.float32)).astype(BF)
    dinv_f = dinv_p.astype(np.float32)
    in_maps = []
    for core in range(c.NC):
        sl = slice(core * c.SHARD, (core + 1) * c.SHARD)
        in_maps.append({
            "g0tab": g0,
            "g0own": np.ascontiguousarray(g0[sl]),
            "W1": np.asarray(W1, dtype=np.float32).astype(BF),
            "W2": np.asarray(W2, dtype=np.float32).astype(BF),
            "b1r": np.tile(np.asarray(b1, np.float32)[None, :], (128, 1)),
            "b2r": np.tile(np.asarray(b2, np.float32)[None, :], (128, 1)),
            "dinvw": np.ascontiguousarray(dinv_f[sl].reshape(c.NW, 128).T),
            "gidx": _wrap_idxs(idx_arr[core]),
            "dstloc": np.ascontiguousarray(
                dstloc_arr[core].reshape(TOT_TILES, 128).T).astype(BF),
            "dstloc32": np.ascontiguousarray(
                dstloc_arr[core].reshape(TOT_TILES, 128).T),
            "didx_s": _wrap_idxs(idx_s[core]),
            "didx_d": _wrap_idxs(idx_d[core]),
        })
    meta = dict(B=B, TPG=TPG, TOT=TOT, TOT_TILES=TOT_TILES,
                B_dec=B_dec, TOT_DEC=TOT_DEC, slot2j=slot2j)
    return in_maps, meta


def build_program(cfg, meta, num_cores=None):
    c = cfg
    NCores = num_cores or c.NC
    B, TPG, TOT, TOT_TILES = meta["B"], meta["TPG"], meta["TOT"], meta["TOT_TILES"]
    B_dec, TOT_DEC = meta["B_dec"], meta["TOT_DEC"]
    D = c.D

    nc = bacc.Bacc("TRN2", target_bir_lowering=False, debug=False,
                   num_devices=NCores, num_swdge_queues=4)
    NQ = 4

    g0_in = nc.dram_tensor("g0tab", [c.NP, D], BF16, kind="ExternalInput")
    g0own_in = nc.dram_tensor("g0own", [c.SHARD, D], BF16, kind="ExternalInput")
    W1_in = nc.dram_tensor("W1", [D, D], BF16, kind="ExternalInput")
    W2_in = nc.dram_tensor("W2", [D, D], BF16, kind="ExternalInput")
    b1_in = nc.dram_tensor("b1r", [128, D], F32, kind="ExternalInput")
    b2_in = nc.dram_tensor("b2r", [128, D], F32, kind="ExternalInput")
    dinvw_in = nc.dram_tensor("dinvw", [128, c.NW], F32, kind="ExternalInput")
    gidx_in = nc.dram_tensor("gidx", [128, TOT // 16], I16, kind="ExternalInput")
    dstloc_in = nc.dram_tensor("dstloc", [128, TOT_TILES], BF16, kind="ExternalInput")
    dstloc32_in = nc.dram_tensor("dstloc32", [128, TOT_TILES], F32, kind="ExternalInput")
    didx_s_in = nc.dram_tensor("didx_s", [128, TOT_DEC // 16], I16, kind="ExternalInput")
    didx_d_in = nc.dram_tensor("didx_d", [128, TOT_DEC // 16], I16, kind="ExternalInput")
    dots_out = nc.dram_tensor("dots", [128, TOT_DEC // 128], F32, kind="ExternalOutput")

    gst = {"count": 0, "prev": None}

    def emit_gather(out_ap, in_ap, idx_ap, n_idx, extra_deps=()):
        q = gst["count"] % NQ
        inst = nc.gpsimd.dma_gather(out_ap, in_ap, idx_ap, n_idx, n_idx, D,
                                    queue_num=q, single_packet=False)
        if gst["prev"] is not None:
            add_dep_helper(inst.ins, gst["prev"].ins, sync=False,
                           reason="pin swdge queue order")
        for dep in extra_deps:
            add_dep_helper(inst.ins, dep.ins, sync=True,
                           reason="gather reads DRAM written by this DMA")
        gst["prev"] = inst
        gst["count"] += 1
        return inst

    shard1 = nc.dram_tensor("shard1", [c.SHARD, D], BF16)
    shardz = nc.dram_tensor("shardz", [c.SHARD, D], BF16)
    table0 = nc.dram_tensor("table0", [c.NP, D], BF16)
    table1 = nc.dram_tensor("table1", [c.NP, D], BF16)
    tablez = nc.dram_tensor("tablez", [c.NP, D], BF16)

    iota_dram = nc.inline_tensor(
        np.tile(np.arange(128, dtype=np.float32), (128, 1)).astype(BF), "iota_c")
    ident_dram = nc.inline_tensor(np.eye(128, dtype=np.float32).astype(BF), "ident_c")

    cc_sem = nc.alloc_semaphore("cc_sem")
    ag_sem1 = nc.alloc_semaphore("ag_sem1")
    ag_sem2 = nc.alloc_semaphore("ag_sem2")
    core_ids = list(range(NCores))

    with tile.TileContext(nc) as tc:
        with contextlib.ExitStack() as es:
            const = es.enter_context(tc.tile_pool(name="const", bufs=1))
            meta_p = es.enter_context(tc.tile_pool(name="meta", bufs=1))

            w1_sb = const.tile([D, D], BF16); nc.sync.dma_start(w1_sb[:], W1_in[:])
            w2_sb = const.tile([D, D], BF16); nc.sync.dma_start(w2_sb[:], W2_in[:])
            b1_sb = const.tile([128, D], F32); nc.sync.dma_start(b1_sb[:], b1_in[:])
            b2_sb = const.tile([128, D], F32); nc.sync.dma_start(b2_sb[:], b2_in[:])
            dinvw_sb = const.tile([128, c.NW], F32)
            nc.sync.dma_start(dinvw_sb[:], dinvw_in[:])
            iota_sb = const.tile([128, 128], BF16)
            nc.sync.dma_start(iota_sb[:], iota_dram[:])
            ident_sb = const.tile([128, 128], BF16)
            nc.sync.dma_start(ident_sb[:], ident_dram[:])
            zeros_sb = const.tile([128, 128], F32)
            nc.vector.memset(zeros_sb[:], 0.0)
            # copy staged g0 to an Internal table unless gathering directly
            t0_copies = []
            if not T0DIRECT:
                if T0CHUNK:
                    step = c.NP // 8
                    for i in range(8):
                        t0_copies.append(nc.sync.dma_start(
                            table0[i * step:(i + 1) * step, :],
                            g0_in[i * step:(i + 1) * step, :]))
                else:
                    t0_copies.append(nc.sync.dma_start(table0[:], g0_in[:]))
            gidx_sb = meta_p.tile([128, TOT // 16], I16)
            nc.sync.dma_start(gidx_sb[:], gidx_in[:])
            dstloc_sb = meta_p.tile([128, TOT_TILES], BF16)
            nc.sync.dma_start(dstloc_sb[:], dstloc_in[:])
            if not S_WIDE:
                dstloc32_sb = meta_p.tile([128, TOT_TILES], F32)
                nc.sync.dma_start(dstloc32_sb[:], dstloc32_in[:])

            def all_gather(shard, table_out, n_before):
                tc.strict_bb_all_engine_barrier()
                with tc.tile_critical():
                    nc.gpsimd.collective_compute(
                        "AllGather", mybir.AluOpType.bypass,
                        replica_groups=[core_ids],
                        ins=[shard[:]], outs=[table_out[:]],
                    ).then_inc(cc_sem)
                    nc.gpsimd.wait_ge(cc_sem, n_before + 1)
                tc.strict_bb_all_engine_barrier()

            # batch index ranges per AG chunk (AGCHUNK chunks over NBATCH)
            def chunk_bounds():
                per = math.ceil(c.NBATCH / AGCHUNK)
                out = []
                for k in range(AGCHUNK):
                    blo, bhi = k * per, min((k + 1) * per, c.NBATCH)
                    if blo < bhi:
                        out.append((blo, bhi))
                return out

            def chain_pool(inst):
                if gst["prev"] is not None:
                    add_dep_helper(inst.ins, gst["prev"].ins, sync=False,
                                   reason="pin pool order")
                gst["prev"] = inst

            def emit_ag_chunk(shard, table_out, shard_sem, k, blo, bhi):
                rlo = blo * c.WB * 128
                rhi = min(bhi * c.WB, c.NW) * 128
                # SP is in-order: drain => all prior shard-row DMAs landed
                tc.no_sync_barrier()
                nc.sync.drain()
                nc.sync.sem_inc(shard_sem, 1)
                chain_pool(nc.gpsimd.wait_ge(shard_sem, k + 1))
                tout = table_out[:].rearrange("(cc r) d -> cc r d", cc=NCores)
                with tc.tile_critical(no_gpsimd_drain=True):
                    inst = nc.gpsimd.collective_compute(
                        "AllGather", mybir.AluOpType.bypass,
                        replica_groups=[core_ids],
                        ins=[shard[rlo:rhi, :]], outs=[tout[:, rlo:rhi, :]],
                    ).then_inc(cc_sem)
                tc.no_sync_barrier()
                return inst

            def make_ag_cb(shard, table_out, shard_sem, bounds):
                done = set()

                def cb(b, shard_dmas):
                    for k, (blo, bhi) in enumerate(bounds):
                        # emit once chunk k's last batch epilogue is 2 old
                        if k not in done and bhi <= b - 1:
                            emit_ag_chunk(shard, table_out, shard_sem, k,
                                          blo, bhi)
                            done.add(k)
                    return None

                def finish(shard_dmas):
                    for k, (blo, bhi) in enumerate(bounds):
                        if k not in done:
                            emit_ag_chunk(shard, table_out, shard_sem, k,
                                          blo, bhi)
                            done.add(k)

                cb.finish = finish
                return cb

            def layer(lid, table, w_sb, bias_sb, outer_dinv, shard_next,
                      own_tiles, own_src, out_pool, first_deps=(),
                      at_batch_start=None, shard_sem=None):
                """own_tiles: list of 98 SBUF [128, D] bf16 tiles holding this
                core's own g rows per window (self-loop term); if None, they
                are DMA-loaded from own_src ([SHARD, D] per-core tensor).
                Returns the produced zt tiles (allocated from out_pool)."""
                out_tiles = []
                with tc.tile_pool(name=f"M{lid}", bufs=3) as Mp, \
                     tc.tile_pool(name=f"S{lid}", bufs=2) as Sp, \
                     tc.tile_pool(name=f"own{lid}", bufs=2 * c.WB) as ownp, \
                     tc.tile_pool(name=f"ag{lid}", bufs=4, space="PSUM") as agp, \
                     tc.tile_pool(name=f"mm{lid}", bufs=2, space="PSUM") as mmp, \
                     tc.tile_pool(name=f"ep{lid}", bufs=3) as epp:
                    shard_dmas = []
                    for b in range(c.NBATCH):
                        if at_batch_start is not None:
                            at_batch_start(b, shard_dmas)
                        wlo = b * c.WB
                        whi = min(wlo + c.WB, c.NW)
                        nwb = whi - wlo
                        cols_per_ch = nwb * TPG
                        ntiles = c.NCH * cols_per_ch
                        slot_base = wlo * c.NCH * B
                        tile_base = slot_base // 128
                        Mt = Mp.tile([128, ntiles, D], BF16, tag="M")
                        for ch in range(c.NCH):
                            n_idx = nwb * B
                            off16 = (slot_base + ch * n_idx) // 16
                            emit_gather(
                                Mt[:, ch * cols_per_ch:(ch + 1) * cols_per_ch, :],
                                table[c.BASES[ch]:c.BASES[ch] + CH_REACH, :],
                                gidx_sb[:, off16:off16 + n_idx // 16],
                                n_idx,
                                extra_deps=(first_deps if b == 0 and ch == 0
                                            else ()))
                        if own_tiles is None:
                            ownts = []
                            for wi in range(nwb):
                                w = wlo + wi
                                ot = ownp.tile([128, D], BF16, tag="own")
                                nc.sync.dma_start(
                                    ot[:], own_src[w * 128:(w + 1) * 128, :])
                                ownts.append(ot)
                        St = Sp.tile([128, ntiles, 128], BF16, tag="S")
                        if S_WIDE:
                            nc.vector.tensor_tensor(
                                St[:],
                                iota_sb[:, None, :].broadcast_to([128, ntiles, 128]),
                                dstloc_sb[:, tile_base:tile_base + ntiles, None]
                                    .broadcast_to([128, ntiles, 128]),
                                op=mybir.AluOpType.is_equal)
                        else:
                            for k in range(ntiles):
                                nc.vector.tensor_scalar(
                                    St[:, k, :], iota_sb[:],
                                    dstloc32_sb[:, tile_base + k:tile_base + k + 1],
                                    None, mybir.AluOpType.is_equal)
                        for wi in range(nwb):
                            w = wlo + wi
                            agg = agp.tile([128, D], F32, tag="agg")
                            nmm = c.NCH * TPG + 1
                            k = 0
                            for ch in range(c.NCH):
                                for t in range(TPG):
                                    mcol = (ch * nwb + wi) * TPG + t
                                    nc.tensor.matmul(
                                        agg[:], lhsT=Mt[:, mcol, :],
                                        rhs=St[:, mcol, :],
                                        start=(k == 0), stop=False)
                                    k += 1
                            self_lhs = (own_tiles[w][:] if own_tiles is not None
                                        else ownts[wi][:])
                            if IDENT:
                                nc.tensor.matmul(agg[:], lhsT=self_lhs,
                                                 rhs=ident_sb[:],
                                                 start=False, stop=True)
                            else:
                                nc.tensor.matmul(agg[:], lhsT=self_lhs,
                                                 rhs=ident_sb[:],
                                                 start=False, stop=True)
                            aggT_sb = epp.tile([128, D], BF16, tag="aggT")
                            if SCOPY:
                                nc.scalar.copy(aggT_sb[:], agg[:])
                            else:
                                nc.vector.tensor_copy(aggT_sb[:], agg[:])
                            mm = mmp.tile([128, D], F32, tag="mm")
                            nc.tensor.matmul(mm[:], lhsT=aggT_sb[:], rhs=w_sb[:],
                                             start=True, stop=True)
                            v = epp.tile([128, D], F32, tag="v")
                            if STT:
                                nc.vector.scalar_tensor_tensor(
                                    v[:], mm[:], dinvw_sb[:, w:w + 1], bias_sb[:],
                                    op0=mybir.AluOpType.mult,
                                    op1=mybir.AluOpType.add)
                            else:
                                nc.vector.tensor_scalar(
                                    v[:], mm[:], dinvw_sb[:, w:w + 1], None,
                                    mybir.AluOpType.mult)
                                nc.vector.tensor_tensor(
                                    v[:], v[:], bias_sb[:],
                                    op=mybir.AluOpType.add)
                            zt = out_pool.tile([128, D], BF16, tag=f"z{lid}")
                            if outer_dinv:
                                if STT:
                                    nc.vector.scalar_tensor_tensor(
                                        zt[:], v[:], dinvw_sb[:, w:w + 1],
                                        zeros_sb[:],
                                        op0=mybir.AluOpType.mult,
                                        op1=mybir.AluOpType.max)
                                else:
                                    rt = epp.tile([128, D], F32, tag="rt")
                                    nc.scalar.activation(
                                        rt[:], v[:],
                                        mybir.ActivationFunctionType.Relu)
                                    nc.vector.tensor_scalar(
                                        zt[:], rt[:], dinvw_sb[:, w:w + 1], None,
                                        mybir.AluOpType.mult)
                            else:
                                nc.scalar.activation(
                                    zt[:], v[:], mybir.ActivationFunctionType.Relu)
                            shard_dmas.append(nc.sync.dma_start(
                                shard_next[w * 128:(w + 1) * 128, :], zt[:]))
                            out_tiles.append(zt)
                return out_tiles, shard_dmas

            l1_table = g0_in if T0DIRECT else table0
            with tc.tile_pool(name="g1keep", bufs=c.NW) as g1p:
                with tc.tile_pool(name="zsink", bufs=3) as zsink:
                    if AGCHUNK > 0:
                        bounds = chunk_bounds()
                        cb1 = make_ag_cb(shard1, table1, ag_sem1, bounds)
                        g1_tiles, sd1 = layer(
                            1, l1_table, w1_sb, b1_sb, True, shard1,
                            None, g0own_in, g1p, first_deps=t0_copies,
                            at_batch_start=cb1, shard_sem=ag_sem1)
                        cb1.finish(sd1)
                        # L2 gathers read table1: gate the gather chain
                        chain_pool(nc.gpsimd.wait_ge(cc_sem, len(bounds)))
                        cb2 = make_ag_cb(shardz, tablez, ag_sem2, bounds)
                        _, sd2 = layer(2, table1, w2_sb, b2_sb, False, shardz,
                                       g1_tiles, None, zsink,
                                       at_batch_start=cb2, shard_sem=ag_sem2)
                        cb2.finish(sd2)
                        chain_pool(nc.gpsimd.wait_ge(cc_sem, 2 * len(bounds)))
                    else:
                        g1_tiles, _ = layer(1, l1_table, w1_sb, b1_sb, True,
                                            shard1, None, g0own_in, g1p,
                                            first_deps=t0_copies)
                        all_gather(shard1, table1, 0)
                        _, _ = layer(2, table1, w2_sb, b2_sb, False, shardz,
                                     g1_tiles, None, zsink)
                        all_gather(shardz, tablez, 1)

            # decode
            DCOLS = TOT_DEC // 128
            with tc.tile_pool(name="didx", bufs=1) as didxp, \
                 tc.tile_pool(name="dM", bufs=1) as dMp, \
                 tc.tile_pool(name="dw", bufs=4) as dwp, \
                 tc.tile_pool(name="dout", bufs=1) as doutp:
                ds_sb = didxp.tile([128, TOT_DEC // 16], I16)
                nc.sync.dma_start(ds_sb[:], didx_s_in[:])
                dd_sb = didxp.tile([128, TOT_DEC // 16], I16)
                nc.sync.dma_start(dd_sb[:], didx_d_in[:])
                Ms = dMp.tile([128, DCOLS, D], BF16, tag="Ms")
                Md = dMp.tile([128, DCOLS, D], BF16, tag="Md")
                res = doutp.tile([128, DCOLS], F32)
                # d-side: one gather per chd (contiguous NCH*B_dec slots)
                for chd in range(c.NCH):
                    n_idx = c.NCH * B_dec
                    off16 = chd * n_idx // 16
                    coff = chd * n_idx // 128
                    emit_gather(
                        Md[:, coff:coff + n_idx // 128, :],
                        tablez[c.BASES[chd]:c.BASES[chd] + CH_REACH, :],
                        dd_sb[:, off16:off16 + n_idx // 16], n_idx)
                # s-side: one gather per (chd, chs) group
                for g in range(c.NCH * c.NCH):
                    chs = g % c.NCH
                    off16 = g * B_dec // 16
                    coff = g * B_dec // 128
                    emit_gather(
                        Ms[:, coff:coff + B_dec // 128, :],
                        tablez[c.BASES[chs]:c.BASES[chs] + CH_REACH, :],
                        ds_sb[:, off16:off16 + B_dec // 16], B_dec)
                for col in range(DCOLS):
                    sc = dwp.tile([128, D], F32, tag="sc")
                    if TTR:
                        nc.vector.tensor_tensor_reduce(
                            out=sc[:], in0=Ms[:, col, :], in1=Md[:, col, :],
                            scale=1.0, scalar=0.0,
                            op0=mybir.AluOpType.mult, op1=mybir.AluOpType.add,
                            accum_out=res[:, col:col + 1])
                    else:
                        nc.vector.tensor_tensor(
                            sc[:], Ms[:, col, :], Md[:, col, :],
                            op=mybir.AluOpType.mult)
                        nc.vector.reduce_sum(res[:, col:col + 1], sc[:],
                                             axis=mybir.AxisListType.X)
                nc.sync.dma_start(dots_out[:], res[:])

    nc.compile()
    return nc


def assemble_output(cfg, meta, results):
    c = cfg
    slot2j = meta["slot2j"]
    out = np.zeros(c.EL, dtype=np.float32)
    for core in range(len(results)):
        d = np.asarray(results[core]["dots"], dtype=np.float32)
        flat = d.T.reshape(-1)             # slot i -> d[i%128, i//128]
        s2j = slot2j[core]
        valid = s2j >= 0
        out[s2j[valid]] = flat[valid]
    return out


def run_pipeline(x, edge_index, edge_label_index, W1, b1, W2, b2,
                 cfg=None, trace=False, tmpdir=None):
    cfg = cfg or DEFAULT
    in_maps, meta = host_prep(cfg, x, edge_index, edge_label_index,
                              W1, b1, W2, b2)
    nc = build_program(cfg, meta)
    res = run_bass_kernel_spmd(nc, in_maps, list(range(cfg.NC)),
                               trace=trace, tmpdir=tmpdir)
    return assemble_output(cfg, meta, res.results), res


def kernel(x, edge_index, edge_label_index, W1, b1, W2, b2):
    out, _ = run_pipeline(x, edge_index, edge_label_index, W1, b1, W2, b2)
    return out
